# revision 2
# baseline (speedup 1.0000x reference)
"""ContrastivePretrainedSAGE Trainium2 kernel v2 (8-core SPMD).

Design: nodes sharded by id range (12544 slots/core = 98 windows of 128).
Edges routed to the dst-owning core, ordered (superwindow, src-group,
window, tile). Source features are fetched with InstDMAGatherAnt
(`dma_gather`): bf16 x rows (512B), 1024 rows per op, two SWDGE queues so
descriptor generation overlaps the previous op's transfer. Segment-sum via
one-hot mask matmuls (mask built on DVE from a slot table), accumulating a
[128,256] f32 PSUM tile per window across the window's 20 tiles (4 groups x
5). Epilogue per window fuses the whole model:
  aggr=(psum)*inv(max(deg,1)); h=relu(aggr@W_l.T+b_l+x@W_r.T);
  gnn=h.w_score + x@(W_res.T@w_score) + (b_res.w_score+b_score);
  out=sig(a)*rer+(1-sig(a))*gnn
with deg precomputed host-side (index-space bincount) and x@[W_r.T|u]
matmuls fed from a resident bf16 x^T slice of the core's own nodes.
"""
import math
from dataclasses import dataclass

import numpy as np
import ml_dtypes

import concourse.bass as bass
import concourse.mybir as mybir
import concourse.tile as tile
from concourse.bass_utils import run_bass_kernel_spmd

F32 = mybir.dt.float32
BF16 = mybir.dt.bfloat16
I16 = mybir.dt.int16
AOP = mybir.AluOpType
ACT = mybir.ActivationFunctionType
NCORE = 8
PAD_SLOT = 255.0


def split_sync_waits(nc) -> int:
    n_split = 0
    for f in nc.m.functions:
        for bb in f.blocks:
            out = []
            changed = False
            for ins in bb.instructions:
                si = ins.sync_info
                waits = list(si.on_wait) if si is not None and si.on_wait else []
                if len(waits) > 1:
                    for g, w in enumerate(waits[:-1]):
                        nop = mybir.InstNoOp(name=f"{ins.name}-waitsplit-{g}")
                        nop.engine = ins.engine
                        nop.sync_info = mybir.SyncInfo(on_wait=[w], on_update=[])
                        out.append(nop)
                    si.on_wait = waits[-1:]
                    changed = True
                    n_split += 1
                out.append(ins)
            if changed:
                bb.instructions.clear()
                for i in out:
                    bb.instructions.append(i)
    return n_split


def finish(nc):
    split_sync_waits(nc)
    import bass_rust
    from concourse.library_config import all_libraries, standard
    m = {}
    for lib in all_libraries:
        for it in lib.instructions:
            m[it] = m.get(it, 0) | (1 << lib.index)
    bass_rust.insert_library_loads(nc, m, len(all_libraries), standard.index)
    mybir.codegen_inst_isa_subclasses(nc)
    return nc


@dataclass
class Cfg:
    nsw: int          # superwindows per core
    bw: int           # windows per superwindow
    kt: int           # tiles per (window, group) run
    nx: int           # padded gather-table rows
    gs: int           # group size (rows per source group, <= 32768)
    ngroups: int = 4
    d_in: int = 256
    d_h: int = 128

    @property
    def wpc(self):
        return self.nsw * self.bw

    @property
    def npc(self):
        return self.wpc * 128

    @property
    def run(self):           # padded rows per (window, group)
        return self.kt * 128

    @property
    def chunk(self):         # rows per (superwindow, group)
        return self.bw * self.run

    @property
    def rows(self):          # gathered rows per core
        return self.wpc * self.ngroups * self.run

    @property
    def ntiles(self):
        return self.rows // 128

    @property
    def ops(self):           # op sizes per (sw, g) chunk
        sizes = []
        left = self.chunk
        while left > 0:
            s = min(1024, left)
            sizes.append(s)
            left -= s
        return sizes


def wrap_idx(idx: np.ndarray) -> np.ndarray:
    """[L] -> [128, L/16] int16 wrapped (i at [i%16, i//16]), replicated 8x."""
    L = len(idx)
    assert L % 16 == 0
    block = np.zeros((16, L // 16), np.int16)
    block[np.arange(L) % 16, np.arange(L) // 16] = idx.astype(np.int16)
    return np.tile(block, (8, 1))


def preprocess(x, edge_index, reranker_scores, cfg: Cfg):
    """Index-space edge routing + pure layout prep of per-core inputs."""
    N = x.shape[0]
    src = np.asarray(edge_index[0], dtype=np.int64)
    dst = np.asarray(edge_index[1], dtype=np.int64)
    xf = np.asarray(x, dtype=np.float32)
    rer = np.asarray(reranker_scores, dtype=np.float32)

    x_pad = np.zeros((cfg.nx, cfg.d_in), np.float32)
    x_pad[:N] = xf
    x_bf = x_pad.astype(ml_dtypes.bfloat16)
    xT_bf = np.ascontiguousarray(x_bf.T)          # [256, nx]

    npc, wpc, run = cfg.npc, cfg.wpc, cfg.run
    g_of = src // cfg.gs

    idx_arr = np.zeros((NCORE, cfg.rows), np.int64)
    slot_arr = np.full((NCORE, cfg.rows), PAD_SLOT, np.float32)
    deg_arr = np.zeros((NCORE, 128, wpc), np.float32)
    rer_arr = np.zeros((NCORE, 128, wpc), np.float32)
    for c in range(NCORE):
        lo = c * npc
        m = (dst >= lo) & (dst < lo + npc)
        s_c = src[m]
        d_c = dst[m] - lo
        g_c = g_of[m]
        w_c = d_c >> 7
        # stream position grouping key: (sw, g, w, arbitrary)
        sw_c = w_c // cfg.bw
        key = (sw_c * cfg.ngroups + g_c) * wpc + w_c
        order = np.argsort(key, kind="stable")
        s_c, d_c, g_c, w_c, key = (a[order] for a in (s_c, d_c, g_c, w_c, key))
        cnt = np.bincount(key, minlength=cfg.nsw * cfg.ngroups * wpc)
        # count of run (w, g) is at key (w//bw*4+g)*wpc + w
        runmax = cnt.max()
        assert runmax <= run, (runmax, run)
        start = np.concatenate([[0], np.cumsum(cnt)[:-1]])
        pos_in_run = np.arange(len(s_c)) - start[key]
        # stream offset of each (sw,g,w) run:
        #  sw * (4*chunk) + g * chunk + (w - sw*bw) * run
        sw_of = w_c // cfg.bw
        base = (sw_of * cfg.ngroups + g_c) * cfg.chunk + (w_c - sw_of * cfg.bw) * run
        pos = base + pos_in_run
        idx_arr[c, pos] = s_c - g_c * cfg.gs
        slot_arr[c, pos] = (d_c & 127).astype(np.float32)
        # padding rows keep idx 0 (valid row in every group), slot PAD_SLOT
        node = lo + np.arange(npc)
        valid = node < N
        dv = np.zeros(npc, np.float32)
        dv[valid] = np.bincount(dst, minlength=N)[node[valid]]
        rv = np.zeros(npc, np.float32)
        rv[valid] = rer[node[valid]]
        deg_arr[c] = dv.reshape(wpc, 128).T
        rer_arr[c] = rv.reshape(wpc, 128).T

    slot_tab = slot_arr.reshape(NCORE, cfg.ntiles, 128).transpose(0, 2, 1)
    slot_tab = np.ascontiguousarray(slot_tab.astype(ml_dtypes.bfloat16))
    idx_wrapped = np.stack([wrap_idx(idx_arr[c]) for c in range(NCORE)])

    xT_own = np.zeros((NCORE, 2, 128, cfg.npc), ml_dtypes.bfloat16)
    for c in range(NCORE):
        lo = c * npc
        hi = min(lo + npc, N)
        xT_own[c, 0, :, :hi - lo] = xT_bf[0:128, lo:hi]
        xT_own[c, 1, :, :hi - lo] = xT_bf[128:256, lo:hi]
    return x_bf, idx_wrapped, slot_tab, deg_arr, rer_arr, xT_own


def build(cfg: Cfg):
    nc = bass.Bass("TRN2", target_bir_lowering=False, debug=False,
                   num_devices=NCORE, dynamic_dma_scratch_size=32768,
                   num_swdge_queues=2)
    D, H = cfg.d_in, cfg.d_h
    wpc, ntiles = cfg.wpc, cfg.ntiles
    xrows = nc.dram_tensor("xrows", [cfg.nx, D], BF16, kind="ExternalInput")
    idx = nc.dram_tensor("idx", [128, cfg.rows // 16], I16, kind="ExternalInput")
    slot = nc.dram_tensor("slot", [128, ntiles], BF16, kind="ExternalInput")
    deg = nc.dram_tensor("deg", [128, wpc], F32, kind="ExternalInput")
    rer = nc.dram_tensor("rer", [128, wpc], F32, kind="ExternalInput")
    xto = nc.dram_tensor("xto", [2, 128, cfg.npc], BF16, kind="ExternalInput")
    w_lT = nc.dram_tensor("w_lT", [D, H], F32, kind="ExternalInput")
    w_rT = nc.dram_tensor("w_rT", [D, H], F32, kind="ExternalInput")
    w_res = nc.dram_tensor("w_res", [H, D], F32, kind="ExternalInput")
    wsc_col = nc.dram_tensor("wsc_col", [H, 1], F32, kind="ExternalInput")
    bres_col = nc.dram_tensor("bres_col", [H, 1], F32, kind="ExternalInput")
    bl_bc = nc.dram_tensor("bl_bc", [128, H], F32, kind="ExternalInput")
    wsc_bc = nc.dram_tensor("wsc_bc", [128, H], F32, kind="ExternalInput")
    iota_bc = nc.dram_tensor("iota_bc", [128, 128], BF16, kind="ExternalInput")
    bscore = nc.dram_tensor("bscore", [1, 1], F32, kind="ExternalInput")
    alpha = nc.dram_tensor("alpha", [1, 1], F32, kind="ExternalInput")
    out = nc.dram_tensor("out", [128, wpc], F32, kind="ExternalOutput")

    op_sizes = cfg.ops

    with tile.TileContext(nc) as tc:
        with (
            tc.tile_pool(name="persist", bufs=1) as pp,
            tc.tile_pool(name="gpool", bufs=6) as gpool,
            tc.tile_pool(name="mpool", bufs=6) as mpool,
            tc.tile_pool(name="wsb", bufs=4) as wsb,
            tc.tile_pool(name="apsum", bufs=(cfg.bw + 1) // 2, space="PSUM") as apsum,
            tc.tile_pool(name="tpsum", bufs=2, space="PSUM") as tpsum,
            tc.tile_pool(name="hpsum", bufs=2, space="PSUM") as hpsum,
        ):
            # ---- persistent loads -------------------------------------
            idx_t = pp.tile([128, cfg.rows // 16], I16)
            nc.sync.dma_start(out=idx_t[:], in_=idx[:])
            slot_t = pp.tile([128, ntiles], BF16)
            nc.sync.dma_start(out=slot_t[:], in_=slot[:])
            deg_t = pp.tile([128, wpc], F32)
            nc.sync.dma_start(out=deg_t[:], in_=deg[:])
            rer_t = pp.tile([128, wpc], F32)
            nc.sync.dma_start(out=rer_t[:], in_=rer[:])
            iota_t = pp.tile([128, 128], BF16)
            nc.sync.dma_start(out=iota_t[:], in_=iota_bc[:])
            blb_t = pp.tile([128, H], F32)
            nc.sync.dma_start(out=blb_t[:], in_=bl_bc[:])
            wscb_t = pp.tile([128, H], F32)
            nc.sync.dma_start(out=wscb_t[:], in_=wsc_bc[:])
            xto_t = []
            for h in range(2):
                t = pp.tile([128, cfg.npc], BF16, tag=f"xto{h}")
                nc.sync.dma_start(out=t[:], in_=xto[h])
                xto_t.append(t)
            wsc_t = pp.tile([H, 1], F32)
            nc.sync.dma_start(out=wsc_t[:], in_=wsc_col[:])
            bres_t = pp.tile([H, 1], F32)
            nc.sync.dma_start(out=bres_t[:], in_=bres_col[:])
            bsc_t = pp.tile([1, 1], F32)
            nc.sync.dma_start(out=bsc_t[:], in_=bscore[:])
            alpha_t = pp.tile([1, 1], F32)
            nc.sync.dma_start(out=alpha_t[:], in_=alpha[:])
            ones_row = pp.tile([1, 128], F32)
            nc.vector.memset(ones_row[:], 1.0)
            out_t = pp.tile([128, wpc], F32)

            # inv degree for all windows
            degc = pp.tile([128, wpc], F32)
            nc.vector.tensor_scalar_max(out=degc[:], in0=deg_t[:], scalar1=1.0)
            invd = pp.tile([128, wpc], F32)
            nc.vector.reciprocal(out=invd[:], in_=degc[:])

            # W_l.T halves -> bf16 [128, H]
            wl_t = []
            for h in range(2):
                tf = pp.tile([128, H], F32, tag=f"wlf{h}")
                nc.sync.dma_start(out=tf[:], in_=w_lT[h * 128:(h + 1) * 128, :])
                t = pp.tile([128, H], BF16, tag=f"wl{h}")
                nc.vector.tensor_copy(out=t[:], in_=tf[:])
                wl_t.append(t)
            # [W_r.T | u] halves -> bf16 [128, H+1]
            wrx_t = []
            for h in range(2):
                tf = pp.tile([128, H], F32, tag=f"wrf{h}")
                nc.sync.dma_start(out=tf[:], in_=w_rT[h * 128:(h + 1) * 128, :])
                t = pp.tile([128, H + 1], BF16, tag=f"wrx{h}")
                nc.vector.tensor_copy(out=t[:, 0:H], in_=tf[:])
                wres_h = pp.tile([H, 128], F32, tag=f"wres{h}")
                nc.sync.dma_start(out=wres_h[:], in_=w_res[:, h * 128:(h + 1) * 128])
                pu = hpsum.tile([128, 1], F32, tag="ph", name="pu")
                nc.tensor.matmul(pu[:], lhsT=wres_h[:], rhs=wsc_t[:],
                                 start=True, stop=True)
                nc.vector.tensor_copy(out=t[:, H:H + 1], in_=pu[:])
                wrx_t.append(t)

            # c = b_res @ w_score + b_score ; a = sigmoid(alpha)
            pc = hpsum.tile([1, 1], F32, tag="ph", name="pc")
            nc.tensor.matmul(pc[:], lhsT=bres_t[:], rhs=wsc_t[:],
                             start=True, stop=True)
            c_t = pp.tile([1, 1], F32)
            nc.vector.tensor_add(out=c_t[:], in0=pc[:], in1=bsc_t[:])
            a_t = pp.tile([1, 1], F32)
            nc.scalar.activation(out=a_t[:], in_=alpha_t[:], func=ACT.Sigmoid)
            oma_t = pp.tile([1, 1], F32)
            nc.vector.tensor_scalar(out=oma_t[:], in0=a_t[:], scalar1=-1.0,
                                    scalar2=1.0, op0=AOP.mult, op1=AOP.add)
            abc_row = pp.tile([1, 3], F32)
            nc.vector.tensor_copy(out=abc_row[:, 0:1], in_=a_t[:])
            nc.vector.tensor_copy(out=abc_row[:, 1:2], in_=oma_t[:])
            nc.vector.tensor_copy(out=abc_row[:, 2:3], in_=c_t[:])
            pbc = hpsum.tile([128, 3], F32, tag="ph", name="pbc")
            nc.tensor.matmul(pbc[:], lhsT=ones_row[:], rhs=abc_row[:],
                             start=True, stop=True)
            abc_t = pp.tile([128, 3], F32)
            nc.vector.tensor_copy(out=abc_t[:], in_=pbc[:])
            a_col, oma_col, c_col = abc_t[:, 0:1], abc_t[:, 1:2], abc_t[:, 2:3]

            # make a bf16 identity for transposes
            ident = pp.tile([128, 128], BF16)
            from concourse.masks import make_identity
            make_identity(nc, ident[:])

            kregs = {}
            for s in set(op_sizes):
                kregs[s] = nc.gpsimd.to_reg(s)

            def epilogue(w, acc):
                aggr = wsb.tile([128, D], BF16, tag="aggr")
                nc.vector.tensor_tensor(
                    out=aggr[:], in0=acc,
                    in1=invd[:, w:w + 1].to_broadcast([128, D]), op=AOP.mult)
                ph = hpsum.tile([128, H + 1], F32, tag="ph")
                for h in range(2):
                    nc.tensor.matmul(
                        ph[:, 0:H + 1],
                        lhsT=xto_t[h][:, w * 128:(w + 1) * 128],
                        rhs=wrx_t[h][:], start=(h == 0), stop=False)
                for h in range(2):
                    pt = tpsum.tile([128, 128], BF16, tag="pt")
                    nc.tensor.transpose(out=pt[:], in_=aggr[:, h * 128:(h + 1) * 128],
                                        identity=ident[:])
                    aggrT = wsb.tile([128, 128], BF16, tag=f"aggrT{h}")
                    nc.vector.tensor_copy(out=aggrT[:], in_=pt[:])
                    nc.tensor.matmul(ph[:, 0:H], lhsT=aggrT[:], rhs=wl_t[h][:],
                                     start=False, stop=(h == 1))
                hpre = wsb.tile([128, H], F32, tag="hpre")
                nc.vector.tensor_add(out=hpre[:], in0=ph[:, 0:H], in1=blb_t[:])
                hrelu = wsb.tile([128, H], F32, tag="hrelu")
                nc.scalar.activation(out=hrelu[:], in_=hpre[:], func=ACT.Relu)
                hw = wsb.tile([128, H], F32, tag="hw")
                nc.vector.tensor_tensor(out=hw[:], in0=hrelu[:], in1=wscb_t[:],
                                        op=AOP.mult)
                gdot = wsb.tile([128, 1], F32, tag="gdot")
                nc.vector.reduce_sum(out=gdot[:], in_=hw[:],
                                     axis=mybir.AxisListType.X)
                g1 = wsb.tile([128, 1], F32, tag="g1")
                nc.vector.tensor_add(out=g1[:], in0=gdot[:], in1=ph[:, H:H + 1])
                g2 = wsb.tile([128, 1], F32, tag="g2")
                nc.vector.tensor_add(out=g2[:], in0=g1[:], in1=c_col)
                g3 = wsb.tile([128, 1], F32, tag="g3")
                nc.vector.tensor_tensor(out=g3[:], in0=g2[:], in1=oma_col,
                                        op=AOP.mult)
                g4 = wsb.tile([128, 1], F32, tag="g4")
                nc.vector.tensor_tensor(out=g4[:], in0=rer_t[:, w:w + 1],
                                        in1=a_col, op=AOP.mult)
                nc.vector.tensor_add(out=out_t[:, w:w + 1], in0=g3[:], in1=g4[:])

            # ---- main loop --------------------------------------------
            opq = 0
            for sw in range(cfg.nsw):
                accs = {}
                for g in range(cfg.ngroups):
                    chunk_tile0 = (sw * cfg.ngroups + g) * (cfg.chunk // 128)
                    pos = 0
                    for osz in op_sizes:
                        nt = osz // 128
                        t0 = chunk_tile0 + pos // 128
                        gb = gpool.tile([128, nt, D], BF16, tag=f"gb{nt}")
                        col0 = (chunk_tile0 * 128 + pos) // 16
                        nc.gpsimd.dma_gather(
                            out_ap=gb[:], in_ap=xrows[g * cfg.gs:(g + 1) * cfg.gs, :],
                            idxs_ap=idx_t[:, col0:col0 + osz // 16],
                            num_idxs=osz, num_idxs_reg=kregs[osz],
                            elem_size=D, queue_num=opq % 2)
                        opq += 1
                        mk = mpool.tile([128, nt, 128], BF16, tag=f"mk{nt}")
                        nc.vector.tensor_tensor(
                            out=mk[:],
                            in0=slot_t[:, t0:t0 + nt].unsqueeze(2)
                                .to_broadcast([128, nt, 128]),
                            in1=iota_t[:].unsqueeze(1).to_broadcast([128, nt, 128]),
                            op=AOP.is_equal)
                        for k in range(nt):
                            t = t0 + k
                            # tile t within chunk: local = t - chunk_tile0
                            loc = t - chunk_tile0
                            wloc = loc // cfg.kt
                            w = sw * cfg.bw + wloc
                            kk = loc % cfg.kt
                            pair, sub = wloc // 2, wloc % 2
                            if g == 0 and kk == 0 and sub == 0:
                                accs[pair] = apsum.tile(
                                    [128, 2 * D], F32, tag="acc",
                                    name=f"accp{pair}")
                            acc = accs[pair][:, sub * D:(sub + 1) * D]
                            last = (g == cfg.ngroups - 1) and (kk == cfg.kt - 1)
                            # start=True zeroes the whole PSUM bank, so only
                            # the pair's very first matmul may set it; the
                            # odd window's region is zeroed by that same
                            # bank-wide start.
                            nc.tensor.matmul(acc, lhsT=mk[:, k, :],
                                             rhs=gb[:, k, :],
                                             start=(g == 0 and kk == 0
                                                    and sub == 0),
                                             stop=last)
                            if last:
                                epilogue(w, acc)
                        pos += osz

            nc.sync.dma_start(out=out[:], in_=out_t[:])

    return finish(nc)


def kernel_impl(x, edge_index, reranker_scores, W_l, b_l, W_r, W_res, b_res,
                w_score, b_score, alpha, trace=False):
    N = int(x.shape[0])
    # 98 windows = 14 superwindows x 7; 12544 slots/core
    cfg = Cfg(nsw=14, bw=7, kt=5, nx=100096, gs=25024)
    assert cfg.npc * NCORE >= N

    x_bf, idx_w, slot_tab, deg_arr, rer_arr, xT_own = preprocess(
        x, edge_index, reranker_scores, cfg)

    common = {
        "xrows": x_bf,
        "w_lT": np.ascontiguousarray(np.asarray(W_l, np.float32).T),
        "w_rT": np.ascontiguousarray(np.asarray(W_r, np.float32).T),
        "w_res": np.asarray(W_res, np.float32),
        "wsc_col": np.asarray(w_score, np.float32).reshape(cfg.d_h, 1),
        "bres_col": np.asarray(b_res, np.float32).reshape(cfg.d_h, 1),
        "bl_bc": np.ascontiguousarray(np.broadcast_to(
            np.asarray(b_l, np.float32), (128, cfg.d_h))),
        "wsc_bc": np.ascontiguousarray(np.broadcast_to(
            np.asarray(w_score, np.float32), (128, cfg.d_h))),
        "iota_bc": np.ascontiguousarray(np.broadcast_to(
            np.arange(128, dtype=np.float32), (128, 128))).astype(
                ml_dtypes.bfloat16),
        "bscore": np.asarray(b_score, np.float32).reshape(1, 1),
        "alpha": np.asarray(alpha, np.float32).reshape(1, 1),
    }
    in_maps = []
    for c in range(NCORE):
        im = dict(common)
        im["idx"] = np.ascontiguousarray(idx_w[c])
        im["slot"] = np.ascontiguousarray(slot_tab[c])
        im["deg"] = np.ascontiguousarray(deg_arr[c])
        im["rer"] = np.ascontiguousarray(rer_arr[c])
        im["xto"] = np.ascontiguousarray(xT_own[c])
        in_maps.append(im)

    nc = build(cfg)
    res = run_bass_kernel_spmd(nc, in_maps, core_ids=list(range(NCORE)),
                               trace=trace)
    pieces = []
    for c in range(NCORE):
        oc = np.asarray(res.results[c]["out"], np.float32)  # [128, wpc]
        flat = oc.T.ravel()
        lo = c * cfg.npc
        pieces.append(flat[:max(0, min(cfg.npc, N - lo))])
    full = np.concatenate(pieces).astype(np.float32)
    return (full, res) if trace else full


def kernel(**inputs):
    out = kernel_impl(
        np.asarray(inputs["x"]),
        np.asarray(inputs["edge_index"]),
        np.asarray(inputs["reranker_scores"]),
        np.asarray(inputs["W_l"]),
        np.asarray(inputs["b_l"]),
        np.asarray(inputs["W_r"]),
        np.asarray(inputs["W_res"]),
        np.asarray(inputs["b_res"]),
        np.asarray(inputs["w_score"]),
        np.asarray(inputs["b_score"]),
        np.asarray(inputs["alpha"]),
    )
    return out.astype(np.float32)


# revision 3
# speedup vs baseline: 1.0583x; 1.0583x over previous
"""ContrastivePretrainedSAGE Trainium2 kernel v2 (8-core SPMD).

Design: nodes sharded by id range (12544 slots/core = 98 windows of 128).
Edges routed to the dst-owning core, ordered (superwindow, src-group,
window, tile). Source features are fetched with InstDMAGatherAnt
(`dma_gather`): bf16 x rows (512B), 1024 rows per op, two SWDGE queues so
descriptor generation overlaps the previous op's transfer. Segment-sum via
one-hot mask matmuls (mask built on DVE from a slot table), accumulating a
[128,256] f32 PSUM tile per window across the window's 20 tiles (4 groups x
5). Epilogue per window fuses the whole model:
  aggr=(psum)*inv(max(deg,1)); h=relu(aggr@W_l.T+b_l+x@W_r.T);
  gnn=h.w_score + x@(W_res.T@w_score) + (b_res.w_score+b_score);
  out=sig(a)*rer+(1-sig(a))*gnn
with deg precomputed host-side (index-space bincount) and x@[W_r.T|u]
matmuls fed from a resident bf16 x^T slice of the core's own nodes.
"""
import math
from dataclasses import dataclass

import numpy as np
import ml_dtypes

import concourse.bass as bass
import concourse.mybir as mybir
import concourse.tile as tile
from concourse.bass_utils import run_bass_kernel_spmd

F32 = mybir.dt.float32
BF16 = mybir.dt.bfloat16
I16 = mybir.dt.int16
AOP = mybir.AluOpType
ACT = mybir.ActivationFunctionType
NCORE = 8
PAD_SLOT = 255.0


def split_sync_waits(nc) -> int:
    n_split = 0
    for f in nc.m.functions:
        for bb in f.blocks:
            out = []
            changed = False
            for ins in bb.instructions:
                si = ins.sync_info
                waits = list(si.on_wait) if si is not None and si.on_wait else []
                if len(waits) > 1:
                    for g, w in enumerate(waits[:-1]):
                        nop = mybir.InstNoOp(name=f"{ins.name}-waitsplit-{g}")
                        nop.engine = ins.engine
                        nop.sync_info = mybir.SyncInfo(on_wait=[w], on_update=[])
                        out.append(nop)
                    si.on_wait = waits[-1:]
                    changed = True
                    n_split += 1
                out.append(ins)
            if changed:
                bb.instructions.clear()
                for i in out:
                    bb.instructions.append(i)
    return n_split


def finish(nc):
    split_sync_waits(nc)
    import bass_rust
    from concourse.library_config import all_libraries, standard
    m = {}
    for lib in all_libraries:
        for it in lib.instructions:
            m[it] = m.get(it, 0) | (1 << lib.index)
    bass_rust.insert_library_loads(nc, m, len(all_libraries), standard.index)
    mybir.codegen_inst_isa_subclasses(nc)
    return nc


@dataclass
class Cfg:
    nsw: int          # superwindows per core
    bw: int           # windows per superwindow
    kt: int           # tiles per (window, group) run
    nx: int           # padded gather-table rows
    gs: int           # group size (rows per source group, <= 32768)
    ngroups: int = 4
    d_in: int = 256
    d_h: int = 128

    @property
    def wpc(self):
        return self.nsw * self.bw

    @property
    def npc(self):
        return self.wpc * 128

    @property
    def run(self):           # padded rows per (window, group)
        return self.kt * 128

    @property
    def chunk(self):         # rows per (superwindow, group)
        return self.bw * self.run

    @property
    def rows(self):          # gathered rows per core
        return self.wpc * self.ngroups * self.run

    @property
    def ntiles(self):
        return self.rows // 128

    @property
    def ops(self):           # op sizes per (sw, g) chunk
        sizes = []
        left = self.chunk
        while left > 0:
            s = min(1024, left)
            sizes.append(s)
            left -= s
        return sizes


def wrap_idx(idx: np.ndarray) -> np.ndarray:
    """[L] -> [128, L/16] int16 wrapped (i at [i%16, i//16]), replicated 8x."""
    L = len(idx)
    assert L % 16 == 0
    block = np.zeros((16, L // 16), np.int16)
    block[np.arange(L) % 16, np.arange(L) // 16] = idx.astype(np.int16)
    return np.tile(block, (8, 1))


def preprocess(x, edge_index, reranker_scores, cfg: Cfg):
    """Index-space edge routing + pure layout prep of per-core inputs."""
    N = x.shape[0]
    src = np.asarray(edge_index[0], dtype=np.int64)
    dst = np.asarray(edge_index[1], dtype=np.int64)
    xf = np.asarray(x, dtype=np.float32)
    rer = np.asarray(reranker_scores, dtype=np.float32)

    x_pad = np.zeros((cfg.nx, cfg.d_in), np.float32)
    x_pad[:N] = xf
    x_bf = x_pad.astype(ml_dtypes.bfloat16)
    xT_bf = np.ascontiguousarray(x_bf.T)          # [256, nx]

    npc, wpc, run = cfg.npc, cfg.wpc, cfg.run
    g_of = src // cfg.gs

    idx_arr = np.zeros((NCORE, cfg.rows), np.int64)
    slot_arr = np.full((NCORE, cfg.rows), PAD_SLOT, np.float32)
    deg_arr = np.zeros((NCORE, 128, wpc), np.float32)
    rer_arr = np.zeros((NCORE, 128, wpc), np.float32)
    for c in range(NCORE):
        lo = c * npc
        m = (dst >= lo) & (dst < lo + npc)
        s_c = src[m]
        d_c = dst[m] - lo
        g_c = g_of[m]
        w_c = d_c >> 7
        # stream position grouping key: (sw, g, w, arbitrary)
        sw_c = w_c // cfg.bw
        key = (sw_c * cfg.ngroups + g_c) * wpc + w_c
        order = np.argsort(key, kind="stable")
        s_c, d_c, g_c, w_c, key = (a[order] for a in (s_c, d_c, g_c, w_c, key))
        cnt = np.bincount(key, minlength=cfg.nsw * cfg.ngroups * wpc)
        # count of run (w, g) is at key (w//bw*4+g)*wpc + w
        runmax = cnt.max()
        assert runmax <= run, (runmax, run)
        start = np.concatenate([[0], np.cumsum(cnt)[:-1]])
        pos_in_run = np.arange(len(s_c)) - start[key]
        # stream offset of each (sw,g,w) run:
        #  sw * (4*chunk) + g * chunk + (w - sw*bw) * run
        sw_of = w_c // cfg.bw
        base = (sw_of * cfg.ngroups + g_c) * cfg.chunk + (w_c - sw_of * cfg.bw) * run
        pos = base + pos_in_run
        idx_arr[c, pos] = s_c - g_c * cfg.gs
        slot_arr[c, pos] = (d_c & 127).astype(np.float32)
        # padding rows keep idx 0 (valid row in every group), slot PAD_SLOT
        node = lo + np.arange(npc)
        valid = node < N
        dv = np.zeros(npc, np.float32)
        dv[valid] = np.bincount(dst, minlength=N)[node[valid]]
        rv = np.zeros(npc, np.float32)
        rv[valid] = rer[node[valid]]
        deg_arr[c] = dv.reshape(wpc, 128).T
        rer_arr[c] = rv.reshape(wpc, 128).T

    slot_tab = slot_arr.reshape(NCORE, cfg.ntiles, 128).transpose(0, 2, 1)
    slot_tab = np.ascontiguousarray(slot_tab.astype(ml_dtypes.bfloat16))
    idx_wrapped = np.stack([wrap_idx(idx_arr[c]) for c in range(NCORE)])

    xT_own = np.zeros((NCORE, 2, 128, cfg.npc), ml_dtypes.bfloat16)
    for c in range(NCORE):
        lo = c * npc
        hi = min(lo + npc, N)
        xT_own[c, 0, :, :hi - lo] = xT_bf[0:128, lo:hi]
        xT_own[c, 1, :, :hi - lo] = xT_bf[128:256, lo:hi]
    return x_bf, idx_wrapped, slot_tab, deg_arr, rer_arr, xT_own


def build(cfg: Cfg):
    nc = bass.Bass("TRN2", target_bir_lowering=False, debug=False,
                   num_devices=NCORE, dynamic_dma_scratch_size=32768,
                   num_swdge_queues=2)
    D, H = cfg.d_in, cfg.d_h
    wpc, ntiles = cfg.wpc, cfg.ntiles
    xrows = nc.dram_tensor("xrows", [cfg.nx, D], BF16, kind="ExternalInput")
    idx = nc.dram_tensor("idx", [128, cfg.rows // 16], I16, kind="ExternalInput")
    slot = nc.dram_tensor("slot", [128, ntiles], BF16, kind="ExternalInput")
    deg = nc.dram_tensor("deg", [128, wpc], F32, kind="ExternalInput")
    rer = nc.dram_tensor("rer", [128, wpc], F32, kind="ExternalInput")
    xto = nc.dram_tensor("xto", [2, 128, cfg.npc], BF16, kind="ExternalInput")
    w_lT = nc.dram_tensor("w_lT", [D, H], F32, kind="ExternalInput")
    w_rT = nc.dram_tensor("w_rT", [D, H], F32, kind="ExternalInput")
    w_res = nc.dram_tensor("w_res", [H, D], F32, kind="ExternalInput")
    wsc_col = nc.dram_tensor("wsc_col", [H, 1], F32, kind="ExternalInput")
    bres_col = nc.dram_tensor("bres_col", [H, 1], F32, kind="ExternalInput")
    bl_bc = nc.dram_tensor("bl_bc", [128, H], F32, kind="ExternalInput")
    wsc_bc = nc.dram_tensor("wsc_bc", [128, H], F32, kind="ExternalInput")
    iota_bc = nc.dram_tensor("iota_bc", [128, 128], BF16, kind="ExternalInput")
    bscore = nc.dram_tensor("bscore", [1, 1], F32, kind="ExternalInput")
    alpha = nc.dram_tensor("alpha", [1, 1], F32, kind="ExternalInput")
    out = nc.dram_tensor("out", [128, wpc], F32, kind="ExternalOutput")

    op_sizes = cfg.ops

    with tile.TileContext(nc) as tc:
        with (
            tc.tile_pool(name="persist", bufs=1) as pp,
            tc.tile_pool(name="gpool", bufs=6) as gpool,
            tc.tile_pool(name="mpool", bufs=6) as mpool,
            tc.tile_pool(name="wsb", bufs=4) as wsb,
            tc.tile_pool(name="apsum", bufs=(cfg.bw + 1) // 2, space="PSUM") as apsum,
            tc.tile_pool(name="tpsum", bufs=2, space="PSUM") as tpsum,
            tc.tile_pool(name="hpsum", bufs=2, space="PSUM") as hpsum,
        ):
            # ---- persistent loads -------------------------------------
            # split the idx load per superwindow so the first gather only
            # waits for its own chunk
            idx_t = pp.tile([128, cfg.rows // 16], I16)
            swcols = cfg.rows // 16 // cfg.nsw
            for s in range(cfg.nsw):
                nc.sync.dma_start(out=idx_t[:, s * swcols:(s + 1) * swcols],
                                  in_=idx[:, s * swcols:(s + 1) * swcols])
            slot_t = pp.tile([128, ntiles], BF16)
            nc.sync.dma_start(out=slot_t[:], in_=slot[:])
            deg_t = pp.tile([128, wpc], F32)
            nc.sync.dma_start(out=deg_t[:], in_=deg[:])
            rer_t = pp.tile([128, wpc], F32)
            nc.sync.dma_start(out=rer_t[:], in_=rer[:])
            iota_t = pp.tile([128, 128], BF16)
            nc.sync.dma_start(out=iota_t[:], in_=iota_bc[:])
            blb_t = pp.tile([128, H], F32)
            nc.sync.dma_start(out=blb_t[:], in_=bl_bc[:])
            wscb_t = pp.tile([128, H], F32)
            nc.sync.dma_start(out=wscb_t[:], in_=wsc_bc[:])
            xto_t = []
            for h in range(2):
                t = pp.tile([128, cfg.npc], BF16, tag=f"xto{h}")
                nc.sync.dma_start(out=t[:], in_=xto[h])
                xto_t.append(t)
            wsc_t = pp.tile([H, 1], F32)
            nc.sync.dma_start(out=wsc_t[:], in_=wsc_col[:])
            bres_t = pp.tile([H, 1], F32)
            nc.sync.dma_start(out=bres_t[:], in_=bres_col[:])
            bsc_t = pp.tile([1, 1], F32)
            nc.sync.dma_start(out=bsc_t[:], in_=bscore[:])
            alpha_t = pp.tile([1, 1], F32)
            nc.sync.dma_start(out=alpha_t[:], in_=alpha[:])
            ones_row = pp.tile([1, 128], F32)
            nc.vector.memset(ones_row[:], 1.0)
            out_t = pp.tile([128, wpc], F32)

            # inv degree for all windows
            degc = pp.tile([128, wpc], F32)
            nc.vector.tensor_scalar_max(out=degc[:], in0=deg_t[:], scalar1=1.0)
            invd = pp.tile([128, wpc], F32)
            nc.vector.reciprocal(out=invd[:], in_=degc[:])

            # W_l.T halves -> bf16 [128, H]
            wl_t = []
            for h in range(2):
                tf = pp.tile([128, H], F32, tag=f"wlf{h}")
                nc.sync.dma_start(out=tf[:], in_=w_lT[h * 128:(h + 1) * 128, :])
                t = pp.tile([128, H], BF16, tag=f"wl{h}")
                nc.vector.tensor_copy(out=t[:], in_=tf[:])
                wl_t.append(t)
            # [W_r.T | u] halves -> bf16 [128, H+1]
            wrx_t = []
            for h in range(2):
                tf = pp.tile([128, H], F32, tag=f"wrf{h}")
                nc.sync.dma_start(out=tf[:], in_=w_rT[h * 128:(h + 1) * 128, :])
                t = pp.tile([128, H + 1], BF16, tag=f"wrx{h}")
                nc.vector.tensor_copy(out=t[:, 0:H], in_=tf[:])
                wres_h = pp.tile([H, 128], F32, tag=f"wres{h}")
                nc.sync.dma_start(out=wres_h[:], in_=w_res[:, h * 128:(h + 1) * 128])
                pu = hpsum.tile([128, 1], F32, tag="ph", name="pu")
                nc.tensor.matmul(pu[:], lhsT=wres_h[:], rhs=wsc_t[:],
                                 start=True, stop=True)
                nc.vector.tensor_copy(out=t[:, H:H + 1], in_=pu[:])
                wrx_t.append(t)

            # c = b_res @ w_score + b_score ; a = sigmoid(alpha)
            pc = hpsum.tile([1, 1], F32, tag="ph", name="pc")
            nc.tensor.matmul(pc[:], lhsT=bres_t[:], rhs=wsc_t[:],
                             start=True, stop=True)
            c_t = pp.tile([1, 1], F32)
            nc.vector.tensor_add(out=c_t[:], in0=pc[:], in1=bsc_t[:])
            a_t = pp.tile([1, 1], F32)
            nc.scalar.activation(out=a_t[:], in_=alpha_t[:], func=ACT.Sigmoid)
            oma_t = pp.tile([1, 1], F32)
            nc.vector.tensor_scalar(out=oma_t[:], in0=a_t[:], scalar1=-1.0,
                                    scalar2=1.0, op0=AOP.mult, op1=AOP.add)
            abc_row = pp.tile([1, 3], F32)
            nc.vector.tensor_copy(out=abc_row[:, 0:1], in_=a_t[:])
            nc.vector.tensor_copy(out=abc_row[:, 1:2], in_=oma_t[:])
            nc.vector.tensor_copy(out=abc_row[:, 2:3], in_=c_t[:])
            pbc = hpsum.tile([128, 3], F32, tag="ph", name="pbc")
            nc.tensor.matmul(pbc[:], lhsT=ones_row[:], rhs=abc_row[:],
                             start=True, stop=True)
            abc_t = pp.tile([128, 3], F32)
            nc.vector.tensor_copy(out=abc_t[:], in_=pbc[:])
            a_col, oma_col, c_col = abc_t[:, 0:1], abc_t[:, 1:2], abc_t[:, 2:3]

            # make a bf16 identity for transposes
            ident = pp.tile([128, 128], BF16)
            from concourse.masks import make_identity
            make_identity(nc, ident[:])

            kregs = {}
            for s in set(op_sizes):
                kregs[s] = nc.gpsimd.to_reg(s)

            def epilogue(w, acc):
                aggr = wsb.tile([128, D], BF16, tag="aggr")
                nc.vector.tensor_tensor(
                    out=aggr[:], in0=acc,
                    in1=invd[:, w:w + 1].to_broadcast([128, D]), op=AOP.mult)
                ph = hpsum.tile([128, H + 1], F32, tag="ph")
                for h in range(2):
                    nc.tensor.matmul(
                        ph[:, 0:H + 1],
                        lhsT=xto_t[h][:, w * 128:(w + 1) * 128],
                        rhs=wrx_t[h][:], start=(h == 0), stop=False)
                for h in range(2):
                    pt = tpsum.tile([128, 128], BF16, tag="pt")
                    nc.tensor.transpose(out=pt[:], in_=aggr[:, h * 128:(h + 1) * 128],
                                        identity=ident[:])
                    aggrT = wsb.tile([128, 128], BF16, tag=f"aggrT{h}")
                    nc.vector.tensor_copy(out=aggrT[:], in_=pt[:])
                    nc.tensor.matmul(ph[:, 0:H], lhsT=aggrT[:], rhs=wl_t[h][:],
                                     start=False, stop=(h == 1))
                hpre = wsb.tile([128, H], F32, tag="hpre")
                nc.vector.tensor_add(out=hpre[:], in0=ph[:, 0:H], in1=blb_t[:])
                hrelu = wsb.tile([128, H], F32, tag="hrelu")
                nc.scalar.activation(out=hrelu[:], in_=hpre[:], func=ACT.Relu)
                hw = wsb.tile([128, H], F32, tag="hw")
                nc.vector.tensor_tensor(out=hw[:], in0=hrelu[:], in1=wscb_t[:],
                                        op=AOP.mult)
                gdot = wsb.tile([128, 1], F32, tag="gdot")
                nc.vector.reduce_sum(out=gdot[:], in_=hw[:],
                                     axis=mybir.AxisListType.X)
                g1 = wsb.tile([128, 1], F32, tag="g1")
                nc.vector.tensor_add(out=g1[:], in0=gdot[:], in1=ph[:, H:H + 1])
                g2 = wsb.tile([128, 1], F32, tag="g2")
                nc.vector.tensor_add(out=g2[:], in0=g1[:], in1=c_col)
                g3 = wsb.tile([128, 1], F32, tag="g3")
                nc.vector.tensor_tensor(out=g3[:], in0=g2[:], in1=oma_col,
                                        op=AOP.mult)
                g4 = wsb.tile([128, 1], F32, tag="g4")
                nc.vector.tensor_tensor(out=g4[:], in0=rer_t[:, w:w + 1],
                                        in1=a_col, op=AOP.mult)
                nc.vector.tensor_add(out=out_t[:, w:w + 1], in0=g3[:], in1=g4[:])

            # ---- main loop --------------------------------------------
            opq = 0
            for sw in range(cfg.nsw):
                accs = {}
                for g in range(cfg.ngroups):
                    chunk_tile0 = (sw * cfg.ngroups + g) * (cfg.chunk // 128)
                    pos = 0
                    for osz in op_sizes:
                        nt = osz // 128
                        t0 = chunk_tile0 + pos // 128
                        gb = gpool.tile([128, nt, D], BF16, tag=f"gb{nt}")
                        col0 = (chunk_tile0 * 128 + pos) // 16
                        nc.gpsimd.dma_gather(
                            out_ap=gb[:], in_ap=xrows[g * cfg.gs:(g + 1) * cfg.gs, :],
                            idxs_ap=idx_t[:, col0:col0 + osz // 16],
                            num_idxs=osz, num_idxs_reg=kregs[osz],
                            elem_size=D, queue_num=opq % 2)
                        opq += 1
                        mk = mpool.tile([128, nt, 128], BF16, tag=f"mk{nt}")
                        nc.vector.tensor_tensor(
                            out=mk[:],
                            in0=slot_t[:, t0:t0 + nt].unsqueeze(2)
                                .to_broadcast([128, nt, 128]),
                            in1=iota_t[:].unsqueeze(1).to_broadcast([128, nt, 128]),
                            op=AOP.is_equal)
                        for k in range(nt):
                            t = t0 + k
                            # tile t within chunk: local = t - chunk_tile0
                            loc = t - chunk_tile0
                            wloc = loc // cfg.kt
                            w = sw * cfg.bw + wloc
                            kk = loc % cfg.kt
                            pair, sub = wloc // 2, wloc % 2
                            if g == 0 and kk == 0 and sub == 0:
                                accs[pair] = apsum.tile(
                                    [128, 2 * D], F32, tag="acc",
                                    name=f"accp{pair}")
                            acc = accs[pair][:, sub * D:(sub + 1) * D]
                            last = (g == cfg.ngroups - 1) and (kk == cfg.kt - 1)
                            # start=True zeroes the whole PSUM bank, so only
                            # the pair's very first matmul may set it; the
                            # odd window's region is zeroed by that same
                            # bank-wide start.
                            nc.tensor.matmul(acc, lhsT=mk[:, k, :],
                                             rhs=gb[:, k, :],
                                             start=(g == 0 and kk == 0
                                                    and sub == 0),
                                             stop=last)
                            if last:
                                epilogue(w, acc)
                        pos += osz

            nc.sync.dma_start(out=out[:], in_=out_t[:])

    return finish(nc)


def kernel_impl(x, edge_index, reranker_scores, W_l, b_l, W_r, W_res, b_res,
                w_score, b_score, alpha, trace=False):
    N = int(x.shape[0])
    # 98 windows = 14 superwindows x 7; 12544 slots/core
    cfg = Cfg(nsw=14, bw=7, kt=5, nx=100096, gs=25024)
    assert cfg.npc * NCORE >= N

    x_bf, idx_w, slot_tab, deg_arr, rer_arr, xT_own = preprocess(
        x, edge_index, reranker_scores, cfg)

    common = {
        "xrows": x_bf,
        "w_lT": np.ascontiguousarray(np.asarray(W_l, np.float32).T),
        "w_rT": np.ascontiguousarray(np.asarray(W_r, np.float32).T),
        "w_res": np.asarray(W_res, np.float32),
        "wsc_col": np.asarray(w_score, np.float32).reshape(cfg.d_h, 1),
        "bres_col": np.asarray(b_res, np.float32).reshape(cfg.d_h, 1),
        "bl_bc": np.ascontiguousarray(np.broadcast_to(
            np.asarray(b_l, np.float32), (128, cfg.d_h))),
        "wsc_bc": np.ascontiguousarray(np.broadcast_to(
            np.asarray(w_score, np.float32), (128, cfg.d_h))),
        "iota_bc": np.ascontiguousarray(np.broadcast_to(
            np.arange(128, dtype=np.float32), (128, 128))).astype(
                ml_dtypes.bfloat16),
        "bscore": np.asarray(b_score, np.float32).reshape(1, 1),
        "alpha": np.asarray(alpha, np.float32).reshape(1, 1),
    }
    in_maps = []
    for c in range(NCORE):
        im = dict(common)
        im["idx"] = np.ascontiguousarray(idx_w[c])
        im["slot"] = np.ascontiguousarray(slot_tab[c])
        im["deg"] = np.ascontiguousarray(deg_arr[c])
        im["rer"] = np.ascontiguousarray(rer_arr[c])
        im["xto"] = np.ascontiguousarray(xT_own[c])
        in_maps.append(im)

    nc = build(cfg)
    res = run_bass_kernel_spmd(nc, in_maps, core_ids=list(range(NCORE)),
                               trace=trace)
    pieces = []
    for c in range(NCORE):
        oc = np.asarray(res.results[c]["out"], np.float32)  # [128, wpc]
        flat = oc.T.ravel()
        lo = c * cfg.npc
        pieces.append(flat[:max(0, min(cfg.npc, N - lo))])
    full = np.concatenate(pieces).astype(np.float32)
    return (full, res) if trace else full


def kernel(**inputs):
    out = kernel_impl(
        np.asarray(inputs["x"]),
        np.asarray(inputs["edge_index"]),
        np.asarray(inputs["reranker_scores"]),
        np.asarray(inputs["W_l"]),
        np.asarray(inputs["b_l"]),
        np.asarray(inputs["W_r"]),
        np.asarray(inputs["W_res"]),
        np.asarray(inputs["b_res"]),
        np.asarray(inputs["w_score"]),
        np.asarray(inputs["b_score"]),
        np.asarray(inputs["alpha"]),
    )
    return out.astype(np.float32)


# revision 5
# speedup vs baseline: 1.0650x; 1.0064x over previous
"""ContrastivePretrainedSAGE Trainium2 kernel v2 (8-core SPMD).

Design: nodes sharded by id range (12544 slots/core = 98 windows of 128).
Edges routed to the dst-owning core, ordered (superwindow, src-group,
window, tile). Source features are fetched with InstDMAGatherAnt
(`dma_gather`): bf16 x rows (512B), 1024 rows per op, two SWDGE queues so
descriptor generation overlaps the previous op's transfer. Segment-sum via
one-hot mask matmuls (mask built on DVE from a slot table), accumulating a
[128,256] f32 PSUM tile per window across the window's 20 tiles (4 groups x
5). Epilogue per window fuses the whole model:
  aggr=(psum)*inv(max(deg,1)); h=relu(aggr@W_l.T+b_l+x@W_r.T);
  gnn=h.w_score + x@(W_res.T@w_score) + (b_res.w_score+b_score);
  out=sig(a)*rer+(1-sig(a))*gnn
with deg precomputed host-side (index-space bincount) and x@[W_r.T|u]
matmuls fed from a resident bf16 x^T slice of the core's own nodes.
"""
import math
from dataclasses import dataclass

import numpy as np
import ml_dtypes

import concourse.bass as bass
import concourse.mybir as mybir
import concourse.tile as tile
from concourse.bass_utils import run_bass_kernel_spmd

F32 = mybir.dt.float32
BF16 = mybir.dt.bfloat16
I16 = mybir.dt.int16
AOP = mybir.AluOpType
ACT = mybir.ActivationFunctionType
NCORE = 8
PAD_SLOT = 255.0


def split_sync_waits(nc) -> int:
    n_split = 0
    for f in nc.m.functions:
        for bb in f.blocks:
            out = []
            changed = False
            for ins in bb.instructions:
                si = ins.sync_info
                waits = list(si.on_wait) if si is not None and si.on_wait else []
                if len(waits) > 1:
                    for g, w in enumerate(waits[:-1]):
                        nop = mybir.InstNoOp(name=f"{ins.name}-waitsplit-{g}")
                        nop.engine = ins.engine
                        nop.sync_info = mybir.SyncInfo(on_wait=[w], on_update=[])
                        out.append(nop)
                    si.on_wait = waits[-1:]
                    changed = True
                    n_split += 1
                out.append(ins)
            if changed:
                bb.instructions.clear()
                for i in out:
                    bb.instructions.append(i)
    return n_split


def finish(nc):
    split_sync_waits(nc)
    import bass_rust
    from concourse.library_config import all_libraries, standard
    m = {}
    for lib in all_libraries:
        for it in lib.instructions:
            m[it] = m.get(it, 0) | (1 << lib.index)
    bass_rust.insert_library_loads(nc, m, len(all_libraries), standard.index)
    mybir.codegen_inst_isa_subclasses(nc)
    return nc


@dataclass
class Cfg:
    nsw: int          # superwindows per core
    bw: int           # windows per superwindow
    kt: int           # tiles per (window, group) run
    nx: int           # padded gather-table rows
    gs: int           # group size (rows per source group, <= 32768)
    ngroups: int = 4
    d_in: int = 256
    d_h: int = 128

    @property
    def wpc(self):
        return self.nsw * self.bw

    @property
    def npc(self):
        return self.wpc * 128

    @property
    def run(self):           # padded rows per (window, group)
        return self.kt * 128

    @property
    def chunk(self):         # rows per (superwindow, group)
        return self.bw * self.run

    @property
    def rows(self):          # gathered rows per core
        return self.wpc * self.ngroups * self.run

    @property
    def ntiles(self):
        return self.rows // 128

    @property
    def ops(self):           # op sizes per (sw, g) chunk
        sizes = []
        left = self.chunk
        while left > 0:
            s = min(1024, left)
            sizes.append(s)
            left -= s
        return sizes


def wrap_idx(idx: np.ndarray) -> np.ndarray:
    """[L] -> [128, L/16] int16 wrapped (i at [i%16, i//16]), replicated 8x."""
    L = len(idx)
    assert L % 16 == 0
    block = np.zeros((16, L // 16), np.int16)
    block[np.arange(L) % 16, np.arange(L) // 16] = idx.astype(np.int16)
    return np.tile(block, (8, 1))


def preprocess(x, edge_index, reranker_scores, cfg: Cfg):
    """Index-space edge routing + pure layout prep of per-core inputs."""
    N = x.shape[0]
    src = np.asarray(edge_index[0], dtype=np.int64)
    dst = np.asarray(edge_index[1], dtype=np.int64)
    xf = np.asarray(x, dtype=np.float32)
    rer = np.asarray(reranker_scores, dtype=np.float32)

    x_pad = np.zeros((cfg.nx, cfg.d_in), np.float32)
    x_pad[:N] = xf
    x_bf = x_pad.astype(ml_dtypes.bfloat16)
    xT_bf = np.ascontiguousarray(x_bf.T)          # [256, nx]

    npc, wpc, run = cfg.npc, cfg.wpc, cfg.run
    g_of = src // cfg.gs

    idx_arr = np.zeros((NCORE, cfg.rows), np.int64)
    slot_arr = np.full((NCORE, cfg.rows), PAD_SLOT, np.float32)
    deg_arr = np.zeros((NCORE, 128, wpc), np.float32)
    rer_arr = np.zeros((NCORE, 128, wpc), np.float32)
    for c in range(NCORE):
        lo = c * npc
        m = (dst >= lo) & (dst < lo + npc)
        s_c = src[m]
        d_c = dst[m] - lo
        g_c = g_of[m]
        w_c = d_c >> 7
        # stream position grouping key: (sw, g, w, arbitrary)
        sw_c = w_c // cfg.bw
        key = (sw_c * cfg.ngroups + g_c) * wpc + w_c
        # sort by src within each run for ascending-address DMA locality
        order = np.argsort(key * (1 << 17) + s_c, kind="stable")
        s_c, d_c, g_c, w_c, key = (a[order] for a in (s_c, d_c, g_c, w_c, key))
        cnt = np.bincount(key, minlength=cfg.nsw * cfg.ngroups * wpc)
        # count of run (w, g) is at key (w//bw*4+g)*wpc + w
        runmax = cnt.max()
        assert runmax <= run, (runmax, run)
        start = np.concatenate([[0], np.cumsum(cnt)[:-1]])
        pos_in_run = np.arange(len(s_c)) - start[key]
        # stream offset of each (sw,g,w) run:
        #  sw * (4*chunk) + g * chunk + (w - sw*bw) * run
        sw_of = w_c // cfg.bw
        base = (sw_of * cfg.ngroups + g_c) * cfg.chunk + (w_c - sw_of * cfg.bw) * run
        pos = base + pos_in_run
        idx_arr[c, pos] = s_c - g_c * cfg.gs
        slot_arr[c, pos] = (d_c & 127).astype(np.float32)
        # padding rows keep idx 0 (valid row in every group), slot PAD_SLOT
        node = lo + np.arange(npc)
        valid = node < N
        dv = np.zeros(npc, np.float32)
        dv[valid] = np.bincount(dst, minlength=N)[node[valid]]
        rv = np.zeros(npc, np.float32)
        rv[valid] = rer[node[valid]]
        deg_arr[c] = dv.reshape(wpc, 128).T
        rer_arr[c] = rv.reshape(wpc, 128).T

    slot_tab = slot_arr.reshape(NCORE, cfg.ntiles, 128).transpose(0, 2, 1)
    slot_tab = np.ascontiguousarray(slot_tab.astype(ml_dtypes.bfloat16))
    idx_wrapped = np.stack([wrap_idx(idx_arr[c]) for c in range(NCORE)])

    xT_own = np.zeros((NCORE, 2, 128, cfg.npc), ml_dtypes.bfloat16)
    for c in range(NCORE):
        lo = c * npc
        hi = min(lo + npc, N)
        xT_own[c, 0, :, :hi - lo] = xT_bf[0:128, lo:hi]
        xT_own[c, 1, :, :hi - lo] = xT_bf[128:256, lo:hi]
    return x_bf, idx_wrapped, slot_tab, deg_arr, rer_arr, xT_own


def build(cfg: Cfg):
    nc = bass.Bass("TRN2", target_bir_lowering=False, debug=False,
                   num_devices=NCORE, dynamic_dma_scratch_size=32768,
                   num_swdge_queues=2)
    D, H = cfg.d_in, cfg.d_h
    wpc, ntiles = cfg.wpc, cfg.ntiles
    xrows = nc.dram_tensor("xrows", [cfg.nx, D], BF16, kind="ExternalInput")
    idx = nc.dram_tensor("idx", [128, cfg.rows // 16], I16, kind="ExternalInput")
    slot = nc.dram_tensor("slot", [128, ntiles], BF16, kind="ExternalInput")
    deg = nc.dram_tensor("deg", [128, wpc], F32, kind="ExternalInput")
    rer = nc.dram_tensor("rer", [128, wpc], F32, kind="ExternalInput")
    xto = nc.dram_tensor("xto", [2, 128, cfg.npc], BF16, kind="ExternalInput")
    w_lT = nc.dram_tensor("w_lT", [D, H], F32, kind="ExternalInput")
    w_rT = nc.dram_tensor("w_rT", [D, H], F32, kind="ExternalInput")
    w_res = nc.dram_tensor("w_res", [H, D], F32, kind="ExternalInput")
    wsc_col = nc.dram_tensor("wsc_col", [H, 1], F32, kind="ExternalInput")
    bres_col = nc.dram_tensor("bres_col", [H, 1], F32, kind="ExternalInput")
    bl_bc = nc.dram_tensor("bl_bc", [128, H], F32, kind="ExternalInput")
    wsc_bc = nc.dram_tensor("wsc_bc", [128, H], F32, kind="ExternalInput")
    iota_bc = nc.dram_tensor("iota_bc", [128, 128], BF16, kind="ExternalInput")
    bscore = nc.dram_tensor("bscore", [1, 1], F32, kind="ExternalInput")
    alpha = nc.dram_tensor("alpha", [1, 1], F32, kind="ExternalInput")
    out = nc.dram_tensor("out", [128, wpc], F32, kind="ExternalOutput")

    op_sizes = cfg.ops

    with tile.TileContext(nc) as tc:
        with (
            tc.tile_pool(name="persist", bufs=1) as pp,
            tc.tile_pool(name="gpool", bufs=6) as gpool,
            tc.tile_pool(name="mpool", bufs=6) as mpool,
            tc.tile_pool(name="wsb", bufs=4) as wsb,
            tc.tile_pool(name="apsum", bufs=(cfg.bw + 1) // 2, space="PSUM") as apsum,
            tc.tile_pool(name="tpsum", bufs=2, space="PSUM") as tpsum,
            tc.tile_pool(name="hpsum", bufs=2, space="PSUM") as hpsum,
        ):
            # ---- persistent loads -------------------------------------
            # load the Q7 gather library up front, overlapping the persist
            # DMAs, so the first dma_gather doesn't stall on it
            from concourse import library_config
            nc.gpsimd.load_library(library_config.mlp)
            # split the idx load per superwindow so the first gather only
            # waits for its own chunk
            idx_t = pp.tile([128, cfg.rows // 16], I16)
            swcols = cfg.rows // 16 // cfg.nsw
            for s in range(cfg.nsw):
                nc.sync.dma_start(out=idx_t[:, s * swcols:(s + 1) * swcols],
                                  in_=idx[:, s * swcols:(s + 1) * swcols])
            slot_t = pp.tile([128, ntiles], BF16)
            nc.sync.dma_start(out=slot_t[:], in_=slot[:])
            deg_t = pp.tile([128, wpc], F32)
            nc.sync.dma_start(out=deg_t[:], in_=deg[:])
            rer_t = pp.tile([128, wpc], F32)
            nc.sync.dma_start(out=rer_t[:], in_=rer[:])
            iota_t = pp.tile([128, 128], BF16)
            nc.sync.dma_start(out=iota_t[:], in_=iota_bc[:])
            blb_t = pp.tile([128, H], F32)
            nc.sync.dma_start(out=blb_t[:], in_=bl_bc[:])
            wscb_t = pp.tile([128, H], F32)
            nc.sync.dma_start(out=wscb_t[:], in_=wsc_bc[:])
            xto_t = []
            for h in range(2):
                t = pp.tile([128, cfg.npc], BF16, tag=f"xto{h}")
                nc.sync.dma_start(out=t[:], in_=xto[h])
                xto_t.append(t)
            wsc_t = pp.tile([H, 1], F32)
            nc.sync.dma_start(out=wsc_t[:], in_=wsc_col[:])
            bres_t = pp.tile([H, 1], F32)
            nc.sync.dma_start(out=bres_t[:], in_=bres_col[:])
            bsc_t = pp.tile([1, 1], F32)
            nc.sync.dma_start(out=bsc_t[:], in_=bscore[:])
            alpha_t = pp.tile([1, 1], F32)
            nc.sync.dma_start(out=alpha_t[:], in_=alpha[:])
            ones_row = pp.tile([1, 128], F32)
            nc.vector.memset(ones_row[:], 1.0)
            out_t = pp.tile([128, wpc], F32)

            # inv degree for all windows
            degc = pp.tile([128, wpc], F32)
            nc.vector.tensor_scalar_max(out=degc[:], in0=deg_t[:], scalar1=1.0)
            invd = pp.tile([128, wpc], F32)
            nc.vector.reciprocal(out=invd[:], in_=degc[:])

            # W_l.T halves -> bf16 [128, H]
            wl_t = []
            for h in range(2):
                tf = pp.tile([128, H], F32, tag=f"wlf{h}")
                nc.sync.dma_start(out=tf[:], in_=w_lT[h * 128:(h + 1) * 128, :])
                t = pp.tile([128, H], BF16, tag=f"wl{h}")
                nc.vector.tensor_copy(out=t[:], in_=tf[:])
                wl_t.append(t)
            # [W_r.T | u] halves -> bf16 [128, H+1]
            wrx_t = []
            for h in range(2):
                tf = pp.tile([128, H], F32, tag=f"wrf{h}")
                nc.sync.dma_start(out=tf[:], in_=w_rT[h * 128:(h + 1) * 128, :])
                t = pp.tile([128, H + 1], BF16, tag=f"wrx{h}")
                nc.vector.tensor_copy(out=t[:, 0:H], in_=tf[:])
                wres_h = pp.tile([H, 128], F32, tag=f"wres{h}")
                nc.sync.dma_start(out=wres_h[:], in_=w_res[:, h * 128:(h + 1) * 128])
                pu = hpsum.tile([128, 1], F32, tag="ph", name="pu")
                nc.tensor.matmul(pu[:], lhsT=wres_h[:], rhs=wsc_t[:],
                                 start=True, stop=True)
                nc.vector.tensor_copy(out=t[:, H:H + 1], in_=pu[:])
                wrx_t.append(t)

            # c = b_res @ w_score + b_score ; a = sigmoid(alpha)
            pc = hpsum.tile([1, 1], F32, tag="ph", name="pc")
            nc.tensor.matmul(pc[:], lhsT=bres_t[:], rhs=wsc_t[:],
                             start=True, stop=True)
            c_t = pp.tile([1, 1], F32)
            nc.vector.tensor_add(out=c_t[:], in0=pc[:], in1=bsc_t[:])
            a_t = pp.tile([1, 1], F32)
            nc.scalar.activation(out=a_t[:], in_=alpha_t[:], func=ACT.Sigmoid)
            oma_t = pp.tile([1, 1], F32)
            nc.vector.tensor_scalar(out=oma_t[:], in0=a_t[:], scalar1=-1.0,
                                    scalar2=1.0, op0=AOP.mult, op1=AOP.add)
            abc_row = pp.tile([1, 3], F32)
            nc.vector.tensor_copy(out=abc_row[:, 0:1], in_=a_t[:])
            nc.vector.tensor_copy(out=abc_row[:, 1:2], in_=oma_t[:])
            nc.vector.tensor_copy(out=abc_row[:, 2:3], in_=c_t[:])
            pbc = hpsum.tile([128, 3], F32, tag="ph", name="pbc")
            nc.tensor.matmul(pbc[:], lhsT=ones_row[:], rhs=abc_row[:],
                             start=True, stop=True)
            abc_t = pp.tile([128, 3], F32)
            nc.vector.tensor_copy(out=abc_t[:], in_=pbc[:])
            a_col, oma_col, c_col = abc_t[:, 0:1], abc_t[:, 1:2], abc_t[:, 2:3]

            # make a bf16 identity for transposes
            ident = pp.tile([128, 128], BF16)
            from concourse.masks import make_identity
            make_identity(nc, ident[:])

            kregs = {}
            for s in set(op_sizes):
                kregs[s] = nc.gpsimd.to_reg(s)

            def epilogue(w, acc):
                aggr = wsb.tile([128, D], BF16, tag="aggr")
                nc.vector.tensor_tensor(
                    out=aggr[:], in0=acc,
                    in1=invd[:, w:w + 1].to_broadcast([128, D]), op=AOP.mult)
                ph = hpsum.tile([128, H + 1], F32, tag="ph")
                for h in range(2):
                    nc.tensor.matmul(
                        ph[:, 0:H + 1],
                        lhsT=xto_t[h][:, w * 128:(w + 1) * 128],
                        rhs=wrx_t[h][:], start=(h == 0), stop=False)
                for h in range(2):
                    pt = tpsum.tile([128, 128], BF16, tag="pt")
                    nc.tensor.transpose(out=pt[:], in_=aggr[:, h * 128:(h + 1) * 128],
                                        identity=ident[:])
                    aggrT = wsb.tile([128, 128], BF16, tag=f"aggrT{h}")
                    nc.vector.tensor_copy(out=aggrT[:], in_=pt[:])
                    nc.tensor.matmul(ph[:, 0:H], lhsT=aggrT[:], rhs=wl_t[h][:],
                                     start=False, stop=(h == 1))
                hpre = wsb.tile([128, H], F32, tag="hpre")
                nc.vector.tensor_add(out=hpre[:], in0=ph[:, 0:H], in1=blb_t[:])
                hrelu = wsb.tile([128, H], F32, tag="hrelu")
                nc.scalar.activation(out=hrelu[:], in_=hpre[:], func=ACT.Relu)
                hw = wsb.tile([128, H], F32, tag="hw")
                nc.vector.tensor_tensor(out=hw[:], in0=hrelu[:], in1=wscb_t[:],
                                        op=AOP.mult)
                gdot = wsb.tile([128, 1], F32, tag="gdot")
                nc.vector.reduce_sum(out=gdot[:], in_=hw[:],
                                     axis=mybir.AxisListType.X)
                g1 = wsb.tile([128, 1], F32, tag="g1")
                nc.vector.tensor_add(out=g1[:], in0=gdot[:], in1=ph[:, H:H + 1])
                g2 = wsb.tile([128, 1], F32, tag="g2")
                nc.vector.tensor_add(out=g2[:], in0=g1[:], in1=c_col)
                g3 = wsb.tile([128, 1], F32, tag="g3")
                nc.vector.tensor_tensor(out=g3[:], in0=g2[:], in1=oma_col,
                                        op=AOP.mult)
                g4 = wsb.tile([128, 1], F32, tag="g4")
                nc.vector.tensor_tensor(out=g4[:], in0=rer_t[:, w:w + 1],
                                        in1=a_col, op=AOP.mult)
                nc.vector.tensor_add(out=out_t[:, w:w + 1], in0=g3[:], in1=g4[:])

            # ---- main loop --------------------------------------------
            opq = 0
            for sw in range(cfg.nsw):
                accs = {}
                for g in range(cfg.ngroups):
                    chunk_tile0 = (sw * cfg.ngroups + g) * (cfg.chunk // 128)
                    pos = 0
                    for osz in op_sizes:
                        nt = osz // 128
                        t0 = chunk_tile0 + pos // 128
                        gb = gpool.tile([128, nt, D], BF16, tag=f"gb{nt}")
                        col0 = (chunk_tile0 * 128 + pos) // 16
                        nc.gpsimd.dma_gather(
                            out_ap=gb[:], in_ap=xrows[g * cfg.gs:(g + 1) * cfg.gs, :],
                            idxs_ap=idx_t[:, col0:col0 + osz // 16],
                            num_idxs=osz, num_idxs_reg=kregs[osz],
                            elem_size=D, queue_num=opq % 2)
                        opq += 1
                        mk = mpool.tile([128, nt, 128], BF16, tag=f"mk{nt}")
                        nc.vector.tensor_tensor(
                            out=mk[:],
                            in0=slot_t[:, t0:t0 + nt].unsqueeze(2)
                                .to_broadcast([128, nt, 128]),
                            in1=iota_t[:].unsqueeze(1).to_broadcast([128, nt, 128]),
                            op=AOP.is_equal)
                        for k in range(nt):
                            t = t0 + k
                            # tile t within chunk: local = t - chunk_tile0
                            loc = t - chunk_tile0
                            wloc = loc // cfg.kt
                            w = sw * cfg.bw + wloc
                            kk = loc % cfg.kt
                            pair, sub = wloc // 2, wloc % 2
                            if g == 0 and kk == 0 and sub == 0:
                                accs[pair] = apsum.tile(
                                    [128, 2 * D], F32, tag="acc",
                                    name=f"accp{pair}")
                            acc = accs[pair][:, sub * D:(sub + 1) * D]
                            last = (g == cfg.ngroups - 1) and (kk == cfg.kt - 1)
                            # start=True zeroes the whole PSUM bank, so only
                            # the pair's very first matmul may set it; the
                            # odd window's region is zeroed by that same
                            # bank-wide start.
                            nc.tensor.matmul(acc, lhsT=mk[:, k, :],
                                             rhs=gb[:, k, :],
                                             start=(g == 0 and kk == 0
                                                    and sub == 0),
                                             stop=last)
                            if last:
                                epilogue(w, acc)
                        pos += osz

            nc.sync.dma_start(out=out[:], in_=out_t[:])

    return finish(nc)


def kernel_impl(x, edge_index, reranker_scores, W_l, b_l, W_r, W_res, b_res,
                w_score, b_score, alpha, trace=False):
    N = int(x.shape[0])
    # 98 windows = 14 superwindows x 7; 12544 slots/core
    cfg = Cfg(nsw=14, bw=7, kt=5, nx=100096, gs=25024)
    assert cfg.npc * NCORE >= N

    x_bf, idx_w, slot_tab, deg_arr, rer_arr, xT_own = preprocess(
        x, edge_index, reranker_scores, cfg)

    common = {
        "xrows": x_bf,
        "w_lT": np.ascontiguousarray(np.asarray(W_l, np.float32).T),
        "w_rT": np.ascontiguousarray(np.asarray(W_r, np.float32).T),
        "w_res": np.asarray(W_res, np.float32),
        "wsc_col": np.asarray(w_score, np.float32).reshape(cfg.d_h, 1),
        "bres_col": np.asarray(b_res, np.float32).reshape(cfg.d_h, 1),
        "bl_bc": np.ascontiguousarray(np.broadcast_to(
            np.asarray(b_l, np.float32), (128, cfg.d_h))),
        "wsc_bc": np.ascontiguousarray(np.broadcast_to(
            np.asarray(w_score, np.float32), (128, cfg.d_h))),
        "iota_bc": np.ascontiguousarray(np.broadcast_to(
            np.arange(128, dtype=np.float32), (128, 128))).astype(
                ml_dtypes.bfloat16),
        "bscore": np.asarray(b_score, np.float32).reshape(1, 1),
        "alpha": np.asarray(alpha, np.float32).reshape(1, 1),
    }
    in_maps = []
    for c in range(NCORE):
        im = dict(common)
        im["idx"] = np.ascontiguousarray(idx_w[c])
        im["slot"] = np.ascontiguousarray(slot_tab[c])
        im["deg"] = np.ascontiguousarray(deg_arr[c])
        im["rer"] = np.ascontiguousarray(rer_arr[c])
        im["xto"] = np.ascontiguousarray(xT_own[c])
        in_maps.append(im)

    nc = build(cfg)
    res = run_bass_kernel_spmd(nc, in_maps, core_ids=list(range(NCORE)),
                               trace=trace)
    pieces = []
    for c in range(NCORE):
        oc = np.asarray(res.results[c]["out"], np.float32)  # [128, wpc]
        flat = oc.T.ravel()
        lo = c * cfg.npc
        pieces.append(flat[:max(0, min(cfg.npc, N - lo))])
    full = np.concatenate(pieces).astype(np.float32)
    return (full, res) if trace else full


def kernel(**inputs):
    out = kernel_impl(
        np.asarray(inputs["x"]),
        np.asarray(inputs["edge_index"]),
        np.asarray(inputs["reranker_scores"]),
        np.asarray(inputs["W_l"]),
        np.asarray(inputs["b_l"]),
        np.asarray(inputs["W_r"]),
        np.asarray(inputs["W_res"]),
        np.asarray(inputs["b_res"]),
        np.asarray(inputs["w_score"]),
        np.asarray(inputs["b_score"]),
        np.asarray(inputs["alpha"]),
    )
    return out.astype(np.float32)
